# revision 44
# baseline (speedup 1.0000x reference)
"""Block-sparse (banded) attention kernel for Trainium2, 8 NeuronCores.

Sharding: data-parallel over batch (2) x tensor-parallel over heads
(16 heads -> 4 per core).  Each core computes its 4 heads' Q/K/V
projections, banded block attention (|r-c| <= 15 blocks, per-block
softmax), and a partial output projection; the host sums the 4 partial
outputs per batch element.

All matmul operands are bf16 (PSUM accumulation stays fp32).  Heads are
processed in pairs per fold; scores / row-sum / broadcast / attn@V
matmuls run as concurrent tile_position'd pairs.  Phase 3 merges the
scores pipeline of unit u with the value pipeline of unit u-1 and the
output projection of the previous slab at macro-step granularity so the
in-order PE queue never chains behind ACT/DVE latency.

Self-contained: hardcodes all shapes; only needs the concourse tree that
the environment already puts on sys.path.
"""

import sys

for _p in ("/opt/trn_rl_repo",):
    if _p not in sys.path:
        sys.path.insert(0, _p)

from contextlib import ExitStack

import numpy as np
import ml_dtypes

import concourse.bacc as bacc
import concourse.tile as tile
from concourse import bass_utils, mybir

F32 = mybir.dt.float32
BF16 = mybir.dt.bfloat16
EXP = mybir.ActivationFunctionType.Exp

B, S, E = 2, 2048, 1024
H, HD, BLK = 16, 64, 64
NB = S // BLK  # 32 blocks
NCORES = 8
HPC = 4  # heads per core
F = HPC * HD  # 256 local features
BAND = 15
SCALE = HD ** -0.5
BFD = ml_dtypes.bfloat16

# per r8-slab (8 query blocks, q=512) column-block ranges, even-extended
T_SLABS = 4
QS = 512  # q extent per slab
LO = []
NP_T = []
for _t in range(T_SLABS):
    lo = max(0, 8 * _t - BAND)
    hi = min(NB - 1, 8 * _t + 7 + BAND)
    if (hi - lo + 1) % 2 == 1:
        if lo > 0:
            lo -= 1
        else:
            hi += 1
    LO.append(lo)
    NP_T.append((hi - lo + 1) // 2)
MAXP = max(NP_T)  # 16 pairs


def build_nc():
    nc = bacc.Bacc("TRN2", target_bir_lowering=False, debug=False)

    xq_d = nc.dram_tensor("xqT", [E, S], BF16, kind="ExternalInput")
    xk_d = nc.dram_tensor("xkT", [E, S], BF16, kind="ExternalInput")
    xv_d = nc.dram_tensor("xvT", [E, S], BF16, kind="ExternalInput")
    wq_d = nc.dram_tensor("wqT", [E, F], BF16, kind="ExternalInput")
    wk_d = nc.dram_tensor("wkT", [E, F], BF16, kind="ExternalInput")
    wv_d = nc.dram_tensor("wvT", [E, F], BF16, kind="ExternalInput")
    wo_d = nc.dram_tensor("woT", [F, E], BF16, kind="ExternalInput")
    sel_d = nc.dram_tensor("selc", [128, MAXP * 32], BF16, kind="ExternalInput")
    vm_d = nc.dram_tensor("vmask", [64, T_SLABS * QS], BF16, kind="ExternalInput")
    out_d = nc.dram_tensor("out", [S, E], F32, kind="ExternalOutput")
    # ping-pong DRAM staging for the reciprocal broadcast (DMA partition
    # replication needs a DRAM source: SBUF APs can't have zero-step
    # partition dims)
    scr_d = [
        nc.dram_tensor(f"rcscr{i}", [64, 512], BF16, kind="Internal") for i in range(2)
    ]

    with tile.TileContext(nc) as tc, ExitStack() as ctx, nc.allow_low_precision(
        reason="bf16 pipeline; fp32 PSUM accumulation throughout"
    ):
        pers = ctx.enter_context(tc.tile_pool(name="pers", bufs=1))
        qT = pers.tile([128, 2 * S], BF16, tag="qT")
        kT = pers.tile([128, 2 * S], BF16, tag="kT")
        vv = pers.tile([128, 16 * F], BF16, tag="vv")
        wq = pers.tile([128, 8 * F], BF16, tag="wq")
        wk = pers.tile([128, 8 * F], BF16, tag="wk")
        wv = pers.tile([128, 8 * F], BF16, tag="wv")
        wo = pers.tile([128, 2 * E], BF16, tag="wo")
        selb = pers.tile([128, MAXP * 32], BF16, tag="selb")
        vm = pers.tile([64, T_SLABS * QS], BF16, tag="vm")

        # k-projection weights first: phase 1 is on the critical path
        nc.sync.dma_start(
            wk[:].rearrange("p (c f) -> p c f", c=8),
            wk_d.ap().rearrange("(c p) f -> p c f", p=128),
        )
        # remaining weights/constants arrive via gpsimd (SWDGE) so they don't
        # queue ahead of the phase-1/2 x-tile loads on the sync ring
        nc.gpsimd.dma_start(
            wv[:].rearrange("p (c f) -> p c f", c=8),
            wv_d.ap().rearrange("(c p) f -> p c f", p=128),
        )
        nc.gpsimd.dma_start(
            wq[:].rearrange("p (c f) -> p c f", c=8),
            wq_d.ap().rearrange("(c p) f -> p c f", p=128),
        )
        nc.gpsimd.dma_start(
            wo[:].rearrange("p (c e) -> p c e", c=2),
            wo_d.ap().rearrange("(c p) e -> p c e", p=128),
        )
        nc.gpsimd.dma_start(selb[:], sel_d.ap())
        nc.gpsimd.dma_start(vm[:], vm_d.ap())

        # big staging tiles for xv/xq: loaded as full [128, 2048] rows (4KB
        # per-partition DMA lines) while phase 1 computes; freed before the
        # attention pools open
        xbig = tc.tile_pool(name="xbig", bufs=1)
        xbp = xbig.__enter__()
        xvAll = xbp.tile([128, 8 * S], BF16, tag="xvAll")
        xqAll = xbp.tile([128, 8 * S], BF16, tag="xqAll")

        # ---- phase 1: k projection (kT layout [f, s]) ----
        with tc.tile_pool(name="xk", bufs=2) as xkp, tc.tile_pool(
            name="psK", bufs=1, space="PSUM"
        ) as pskp:
            psK = pskp.tile([128, 4096], F32)
            for e in range(8):
                xt = xkp.tile([128, S], BF16, tag="xk")
                nc.sync.dma_start(xt[:], xk_d.ap()[e * 128 : (e + 1) * 128, :])
                # xv/xq stream on the scalar/vector queues so the sync ring
                # feeds phase-1 xk tiles back-to-back
                nc.scalar.dma_start(
                    xvAll[:, e * S : (e + 1) * S], xv_d.ap()[e * 128 : (e + 1) * 128, :]
                )
                nc.scalar.dma_start(
                    xqAll[:, e * S : (e + 1) * S], xq_d.ap()[e * 128 : (e + 1) * 128, :]
                )
                for fold in range(2):
                    for sc in range(4):
                        nc.tensor.matmul(
                            psK[:, (fold * 4 + sc) * 512 : (fold * 4 + sc + 1) * 512],
                            wk[:, e * F + fold * 128 : e * F + fold * 128 + 128],
                            xt[:, sc * 512 : (sc + 1) * 512],
                            start=(e == 0),
                            stop=(e == 7),
                        )
            for fold in range(2):
                for sc in range(4):
                    nc.vector.tensor_copy(
                        kT[:, fold * S + sc * 512 : fold * S + (sc + 1) * 512],
                        psK[:, (fold * 4 + sc) * 512 : (fold * 4 + sc + 1) * 512],
                    )

        # ---- phase 2: v projection (natural layout [s, f]) ----
        # x already staged in xvAll; one accumulation group per PSUM bank
        with tc.tile_pool(name="psV", bufs=2, space="PSUM") as psvp:
            for sc in range(4):
                pvs = [
                    psvp.tile([128, 256], F32, name=f"pv{sub}", tag=f"psV{sub}")
                    for sub in range(4)
                ]
                for e in range(8):
                    for sub in range(4):
                        nc.tensor.matmul(
                            pvs[sub][:],
                            xvAll[:, e * S + sc * 512 + sub * 128 : e * S + sc * 512 + (sub + 1) * 128],
                            wv[:, e * F : (e + 1) * F],
                            start=(e == 0),
                            stop=(e == 7),
                        )
                for sub in range(4):
                    nc.vector.tensor_copy(
                        vv[:, sc * 1024 + sub * 256 : sc * 1024 + (sub + 1) * 256],
                        pvs[sub][:],
                    )

        # ---- phase 2.5: q projection for all four slabs ----
        with tc.tile_pool(name="psQ", bufs=2, space="PSUM") as psqp:
            for sc4 in range(4):
                pqs = [
                    psqp.tile([128, 512], F32, name=f"pq{fold}", tag=f"psQ{fold}")
                    for fold in range(2)
                ]
                for e in range(8):
                    for fold in range(2):
                        nc.tensor.matmul(
                            pqs[fold][:],
                            wq[:, e * F + fold * 128 : e * F + fold * 128 + 128],
                            xqAll[:, e * S + sc4 * 512 : e * S + (sc4 + 1) * 512],
                            start=(e == 0),
                            stop=(e == 7),
                        )
                for fold in range(2):
                    nc.scalar.copy(
                        qT[:, fold * S + sc4 * 512 : fold * S + (sc4 + 1) * 512],
                        pqs[fold][:],
                    )
        xbig.__exit__(None, None, None)

        # ---- phase 3: merged attention pipeline + output projection ----
        # PSUM budget (8 banks): psH [128,512]x5 = 5, po x1 = 1, accs x1 = 1,
        # acco x1 = 1.  All cross-engine consumers lag their producers by two
        # j-steps so the PE issue stream never carries an unsatisfied wait.
        # The reciprocal broadcast goes through DRAM + replicating DMAs on the
        # gpsimd SWDGE queue (FIFO-ordered), so the probability multiplies are
        # all-bf16 SBUF ops (DVE 2x mode) and the PE sheds the 224 broadcast
        # matmuls.
        psHp = ctx.enter_context(tc.tile_pool(name="psH", bufs=2, space="PSUM"))
        pop = ctx.enter_context(tc.tile_pool(name="poP", bufs=1, space="PSUM"))
        accp = ctx.enter_context(tc.tile_pool(name="accP", bufs=2, space="PSUM"))
        accop = ctx.enter_context(tc.tile_pool(name="accoP", bufs=1, space="PSUM"))
        expp = ctx.enter_context(tc.tile_pool(name="expS", bufs=4))
        btap = ctx.enter_context(tc.tile_pool(name="btA", bufs=2))
        ptp = ctx.enter_context(tc.tile_pool(name="pt", bufs=6))
        rcpp = ctx.enter_context(tc.tile_pool(name="rcp", bufs=2))
        attp = ctx.enter_context(tc.tile_pool(name="att", bufs=4))
        outp = ctx.enter_context(tc.tile_pool(name="outsb", bufs=2))

        def bcast_dma(scr, bta, h, g, j0, nj):
            """Replicate scr rows 32h+2j+g (j in [j0,j0+nj)) across dst
            partition group g of bta, one row per 512-wide j column block."""
            src = (
                scr.ap()[32 * h : 32 * h + 32, :]
                .rearrange("(j g) c -> g j c", g=2)[g][j0 : j0 + nj]
                .unsqueeze(0)
                .to_broadcast([64, nj, 512])
            )
            dst = bta[g * 64 : (g + 1) * 64, j0 * 512 : (j0 + nj) * 512].rearrange(
                "p (j c) -> p j c", c=512
            )
            nc.sync.dma_start(dst, src)

        def stream_A(t, f, ui):
            """Scores + exp + row sums for head pair (2f, 2f+1) of slab t,
            ending with the reciprocal + DRAM bounce + first broadcast DMAs.

            Returns (steps, shared) where shared collects the tiles stream_B
            needs.  Row sums lag scores by one j-step.
            """
            npt = NP_T[t]
            lo = LO[t]
            scr = scr_d[ui % 2]
            shared = {}

            def alloc(_k=0):
                shared["expS0"] = expp.tile([128, MAXP * QS], BF16, name="expS0", tag="expS")
                shared["expS1"] = expp.tile([128, MAXP * QS], BF16, name="expS1", tag="expS")

            def scores(k):
                # one macro = j pair (2k, 2k+1); one [128,1024] exp per head
                if k == 0:
                    alloc()
                for h, base in ((0, 0), (1, 64)):
                    ps = psHp.tile([128, 1024], F32, name="psH", tag="psH")
                    for jj in range(2):
                        c0 = lo + 2 * (2 * k + jj)
                        nc.tensor.matmul(
                            ps[:, jj * 512 : (jj + 1) * 512],
                            kT[base : base + 64, f * S + c0 * 64 : f * S + c0 * 64 + 128],
                            qT[base : base + 64, f * S + t * QS : f * S + (t + 1) * QS],
                            start=True,
                            stop=True,
                        )
                    nc.scalar.activation(
                        shared["expS0" if h == 0 else "expS1"][
                            :, 2 * k * QS : (2 * k + 2) * QS
                        ],
                        ps[:],
                        EXP,
                    )

            def rowsums(j):
                if j == 0:
                    shared["accs"] = accp.tile([128, 512], F32, name="accs", tag="accP")
                accs = shared["accs"]
                for h, expS in ((0, shared["expS0"]), (1, shared["expS1"])):
                    nc.tensor.matmul(
                        accs[32 * h : 32 * h + 32, :],
                        selb[:, j * 32 : (j + 1) * 32],
                        expS[:, j * QS : (j + 1) * QS],
                        start=(j == 0),
                        stop=(j == npt - 1),
                        skip_group_check=True,
                    )

            def recip_bounce():
                rowsums(npt - 2)
                rowsums(npt - 1)
                rc = rcpp.tile([64, 512], BF16, name="rc", tag="rcp")
                rs1 = rcpp.tile([64, 512], F32, name="rs1", tag="rcs1", bufs=1)
                rs2 = rcpp.tile([64, 512], F32, name="rs2", tag="rcs2", bufs=1)
                nc.vector.reciprocal_approx_accurate(rs2[:], shared["accs"][0:64, :], rs1[:])
                nc.vector.tensor_mul(rc[:], rs2[:], vm[:, t * QS : (t + 1) * QS])
                nc.sync.dma_start(scr.ap(), rc[:])
                shared["bt"] = [
                    btap.tile([128, MAXP * QS], BF16, name=f"bta{h}", tag=f"btA{h}")
                    for h in range(2)
                ]
                # issue the full broadcast now; the gpsimd queue drains it
                # while the next unit's scores run
                for h in range(2):
                    for g in range(2):
                        bcast_dma(scr, shared["bt"][h], h, g, 0, min(4, npt))
                for h in range(2):
                    for g in range(2):
                        bcast_dma(scr, shared["bt"][h], h, g, 4, npt - 4)

            mA = npt // 2
            steps = []
            for k in range(mA):
                steps.append(
                    lambda k=k: (
                        scores(k),
                        k > 0 and (rowsums(2 * k - 2), rowsums(2 * k - 1)),
                    )
                )
            steps.append(recip_bounce)
            return steps, shared

        def stream_B(t, f, ui, shared, attn_t):
            """Probability multiplies + attn@V for the unit A just finished.

            The reciprocal broadcast tensors arrive by DMA (issued in A's tail
            and in early B steps); pt-muls are all-bf16 SBUF DVE ops.  V
            matmuls lag the pt-mul by two j-steps.
            """
            npt = NP_T[t]
            lo = LO[t]
            scr = scr_d[ui % 2]
            st = {}

            def ptmul(k):
                pts = []
                for h in range(2):
                    expS = shared["expS0" if h == 0 else "expS1"]
                    ptt = ptp.tile([128, 1024], BF16, name="ptt", tag="pt")
                    nc.vector.tensor_mul(
                        ptt[:],
                        expS[:, 2 * k * QS : (2 * k + 2) * QS],
                        shared["bt"][h][:, 2 * k * QS : (2 * k + 2) * QS],
                    )
                    pts.append(ptt)
                st[("pt", k)] = pts

            def vmm(k):
                if k == 0:
                    st["acco"] = accop.tile([128, 512], F32, name="acco", tag="accoP")
                acco = st["acco"]
                pts = st.pop(("pt", k))
                for jj in range(2):
                    j = 2 * k + jj
                    cp = lo // 2 + j
                    for h in range(2):
                        nc.tensor.matmul(
                            acco[64 * h : 64 * h + 64, :],
                            vv[:, cp * F + (2 * f + h) * 64 : cp * F + (2 * f + h) * 64 + 64],
                            pts[h][:, jj * 512 : (jj + 1) * 512],
                            start=(j == 0),
                            stop=(j == npt - 1),
                            skip_group_check=True,
                        )

            mB = npt // 2
            steps = []
            for k in range(mB):
                steps.append(lambda k=k: (ptmul(k), k > 1 and vmm(k - 2)))
            steps.append(lambda: vmm(mB - 2))
            steps.append(lambda: (vmm(mB - 1), nc.vector.tensor_copy(attn_t[:], st["acco"][:])))
            return steps

        def stream_C(t, atts, pad=True):
            """Output projection of slab t, dripped 2 matmuls per macro step."""
            st = {}

            def piece(sc2, eh):
                if eh == 0:
                    st["ob"] = outp.tile([128, 1024], F32, name="ob", tag="outsb")
                po = pop.tile([128, 512], F32, name="po", tag="poP")
                for f in range(2):
                    nc.tensor.matmul(
                        po[:],
                        atts[f][:, sc2 * 128 : sc2 * 128 + 128],
                        wo[:, f * E + eh * 512 : f * E + eh * 512 + 512],
                        start=(f == 0),
                        stop=(f == 1),
                    )
                nc.vector.tensor_copy(st["ob"][:, eh * 512 : (eh + 1) * 512], po[:])
                if eh == 1:
                    row = (4 * t + sc2) * 128
                    nc.sync.dma_start(out_d.ap()[row : row + 128, :], st["ob"][:])

            steps = []
            for sc2 in range(4):
                for eh in range(2):
                    steps.append(lambda sc2=sc2, eh=eh: piece(sc2, eh))
                    if pad:
                        steps.append(lambda: None)  # half rate: po shares btP slots
            return steps

        def merge(streams):
            for k in range(max(len(s) for s in streams)):
                for s in streams:
                    if k < len(s):
                        s[k]()

        units = [(t, f) for t in range(T_SLABS) for f in range(2)]
        atts_by_t = {t: [] for t in range(T_SLABS)}
        pending_B = None
        for i, (t, f) in enumerate(units):
            sA, shared = stream_A(t, f, i)
            streams = [sA]
            if pending_B is not None:
                streams.append(pending_B)
            if f == 1 and t >= 1:
                streams.append(stream_C(t - 1, atts_by_t[t - 1], pad=False))
            merge(streams)
            attn_t = attp.tile([128, 512], BF16, name="attn_t", tag="att")
            atts_by_t[t].append(attn_t)
            pending_B = stream_B(t, f, i, shared, attn_t)
        merge([pending_B])
        merge([stream_C(T_SLABS - 1, atts_by_t[T_SLABS - 1], pad=False)])

    nc.compile()
    return nc


_NC_CACHE = []


def _get_nc():
    if not _NC_CACHE:
        _NC_CACHE.append(build_nc())
    return _NC_CACHE[0]


def _host_consts():
    selc = np.zeros((128, MAXP * 32), np.float32)
    for k in range(128):
        for j in range(MAXP):
            selc[k, j * 32 + 2 * j + k // 64] = 1.0
    vmask = np.zeros((64, T_SLABS * QS), np.float32)
    for t in range(T_SLABS):
        for m in range(2 * NP_T[t]):
            c = LO[t] + m
            for qb in range(QS // BLK):
                r = 8 * t + qb
                if abs(r - c) <= BAND:
                    vmask[m, t * QS + qb * 64 : t * QS + (qb + 1) * 64] = 1.0
                    vmask[32 + m, t * QS + qb * 64 : t * QS + (qb + 1) * 64] = 1.0
    return selc.astype(BFD), vmask.astype(BFD)


def build_in_maps(query, key, value, Wq, Wk, Wv, Wo):
    query = np.asarray(query, np.float32)
    key = np.asarray(key, np.float32)
    value = np.asarray(value, np.float32)
    Wq = np.asarray(Wq, np.float32)
    Wk = np.asarray(Wk, np.float32)
    Wv = np.asarray(Wv, np.float32)
    Wo = np.asarray(Wo, np.float32)

    selc, vmask = _host_consts()
    xs = [np.ascontiguousarray(a[b].T).astype(BFD) for a in (query, key, value) for b in range(B)]
    in_maps = []
    for c in range(NCORES):
        b, g = divmod(c, HPC)
        fs = slice(F * g, F * (g + 1))
        in_maps.append(
            {
                "xqT": xs[0 + b],
                "xkT": xs[2 + b],
                "xvT": xs[4 + b],
                "wqT": np.ascontiguousarray((Wq[fs, :] * SCALE).T).astype(BFD),
                "wkT": np.ascontiguousarray(Wk[fs, :].T).astype(BFD),
                "wvT": np.ascontiguousarray(Wv[fs, :].T).astype(BFD),
                "woT": np.ascontiguousarray(Wo[:, fs].T).astype(BFD),
                "selc": selc,
                "vmask": vmask,
            }
        )
    return in_maps


def kernel(query, key, value, Wq, Wk, Wv, Wo):
    nc = _get_nc()
    in_maps = build_in_maps(query, key, value, Wq, Wk, Wv, Wo)
    res = bass_utils.run_bass_kernel_spmd(nc, in_maps, core_ids=list(range(NCORES)))
    out = np.zeros((B, S, E), np.float32)
    for c in range(NCORES):
        b = c // HPC
        out[b] += res.results[c]["out"]
    return out


# revision 45
# speedup vs baseline: 1.0082x; 1.0082x over previous
"""Block-sparse (banded) attention kernel for Trainium2, 8 NeuronCores.

Sharding: data-parallel over batch (2) x tensor-parallel over heads
(16 heads -> 4 per core).  Each core computes its 4 heads' Q/K/V
projections, banded block attention (|r-c| <= 15 blocks, per-block
softmax), and a partial output projection; the host sums the 4 partial
outputs per batch element.

All matmul operands are bf16 (PSUM accumulation stays fp32).  Heads are
processed in pairs per fold; scores / row-sum / broadcast / attn@V
matmuls run as concurrent tile_position'd pairs.  Phase 3 merges the
scores pipeline of unit u with the value pipeline of unit u-1 and the
output projection of the previous slab at macro-step granularity so the
in-order PE queue never chains behind ACT/DVE latency.

Self-contained: hardcodes all shapes; only needs the concourse tree that
the environment already puts on sys.path.
"""

import sys

for _p in ("/opt/trn_rl_repo",):
    if _p not in sys.path:
        sys.path.insert(0, _p)

from contextlib import ExitStack

import numpy as np
import ml_dtypes

import concourse.bacc as bacc
import concourse.tile as tile
from concourse import bass_utils, mybir

F32 = mybir.dt.float32
BF16 = mybir.dt.bfloat16
EXP = mybir.ActivationFunctionType.Exp

B, S, E = 2, 2048, 1024
H, HD, BLK = 16, 64, 64
NB = S // BLK  # 32 blocks
NCORES = 8
HPC = 4  # heads per core
F = HPC * HD  # 256 local features
BAND = 15
SCALE = HD ** -0.5
BFD = ml_dtypes.bfloat16

# per r8-slab (8 query blocks, q=512) column-block ranges, even-extended
T_SLABS = 4
QS = 512  # q extent per slab
LO = []
NP_T = []
for _t in range(T_SLABS):
    lo = max(0, 8 * _t - BAND)
    hi = min(NB - 1, 8 * _t + 7 + BAND)
    if (hi - lo + 1) % 2 == 1:
        if lo > 0:
            lo -= 1
        else:
            hi += 1
    LO.append(lo)
    NP_T.append((hi - lo + 1) // 2)
MAXP = max(NP_T)  # 16 pairs


def build_nc():
    nc = bacc.Bacc("TRN2", target_bir_lowering=False, debug=False)

    xq_d = nc.dram_tensor("xqT", [E, S], BF16, kind="ExternalInput")
    xk_d = nc.dram_tensor("xkT", [E, S], BF16, kind="ExternalInput")
    xv_d = nc.dram_tensor("xvT", [E, S], BF16, kind="ExternalInput")
    wq_d = nc.dram_tensor("wqT", [E, F], BF16, kind="ExternalInput")
    wk_d = nc.dram_tensor("wkT", [E, F], BF16, kind="ExternalInput")
    wv_d = nc.dram_tensor("wvT", [E, F], BF16, kind="ExternalInput")
    wo_d = nc.dram_tensor("woT", [F, E], BF16, kind="ExternalInput")
    sel_d = nc.dram_tensor("selc", [128, MAXP * 32], BF16, kind="ExternalInput")
    vm_d = nc.dram_tensor("vmask", [64, T_SLABS * QS], BF16, kind="ExternalInput")
    out_d = nc.dram_tensor("out", [S, E], F32, kind="ExternalOutput")
    # ping-pong DRAM staging for the reciprocal broadcast (DMA partition
    # replication needs a DRAM source: SBUF APs can't have zero-step
    # partition dims)
    scr_d = [
        nc.dram_tensor(f"rcscr{i}", [64, 512], BF16, kind="Internal") for i in range(2)
    ]

    with tile.TileContext(nc) as tc, ExitStack() as ctx, nc.allow_low_precision(
        reason="bf16 pipeline; fp32 PSUM accumulation throughout"
    ):
        pers = ctx.enter_context(tc.tile_pool(name="pers", bufs=1))
        qT = pers.tile([128, 2 * S], BF16, tag="qT")
        kT = pers.tile([128, 2 * S], BF16, tag="kT")
        vv = pers.tile([128, 16 * F], BF16, tag="vv")
        wq = pers.tile([128, 8 * F], BF16, tag="wq")
        wk = pers.tile([128, 8 * F], BF16, tag="wk")
        wv = pers.tile([128, 8 * F], BF16, tag="wv")
        wo = pers.tile([128, 2 * E], BF16, tag="wo")
        selb = pers.tile([128, MAXP * 32], BF16, tag="selb")
        vm = pers.tile([64, T_SLABS * QS], BF16, tag="vm")

        # k-projection weights first: phase 1 is on the critical path
        nc.sync.dma_start(
            wk[:].rearrange("p (c f) -> p c f", c=8),
            wk_d.ap().rearrange("(c p) f -> p c f", p=128),
        )
        # remaining weights/constants arrive via gpsimd (SWDGE) so they don't
        # queue ahead of the phase-1/2 x-tile loads on the sync ring
        nc.gpsimd.dma_start(
            wv[:].rearrange("p (c f) -> p c f", c=8),
            wv_d.ap().rearrange("(c p) f -> p c f", p=128),
        )
        nc.gpsimd.dma_start(
            wq[:].rearrange("p (c f) -> p c f", c=8),
            wq_d.ap().rearrange("(c p) f -> p c f", p=128),
        )
        nc.gpsimd.dma_start(
            wo[:].rearrange("p (c e) -> p c e", c=2),
            wo_d.ap().rearrange("(c p) e -> p c e", p=128),
        )
        nc.gpsimd.dma_start(selb[:], sel_d.ap())
        nc.gpsimd.dma_start(vm[:], vm_d.ap())

        # big staging tiles for xv/xq: loaded as full [128, 2048] rows (4KB
        # per-partition DMA lines) while phase 1 computes; freed before the
        # attention pools open
        xbig = tc.tile_pool(name="xbig", bufs=1)
        xbp = xbig.__enter__()
        xvAll = xbp.tile([128, 8 * S], BF16, tag="xvAll")
        xqAll = xbp.tile([128, 8 * S], BF16, tag="xqAll")

        # ---- phase 1: k projection (kT layout [f, s]) ----
        with tc.tile_pool(name="xk", bufs=2) as xkp, tc.tile_pool(
            name="psK", bufs=1, space="PSUM"
        ) as pskp:
            psK = pskp.tile([128, 4096], F32)
            for e in range(8):
                xt = xkp.tile([128, S], BF16, tag="xk")
                nc.sync.dma_start(xt[:], xk_d.ap()[e * 128 : (e + 1) * 128, :])
                # xv/xq stream on the scalar/vector queues so the sync ring
                # feeds phase-1 xk tiles back-to-back
                nc.scalar.dma_start(
                    xvAll[:, e * S : (e + 1) * S], xv_d.ap()[e * 128 : (e + 1) * 128, :]
                )
                nc.scalar.dma_start(
                    xqAll[:, e * S : (e + 1) * S], xq_d.ap()[e * 128 : (e + 1) * 128, :]
                )
                for fold in range(2):
                    for sc in range(4):
                        nc.tensor.matmul(
                            psK[:, (fold * 4 + sc) * 512 : (fold * 4 + sc + 1) * 512],
                            wk[:, e * F + fold * 128 : e * F + fold * 128 + 128],
                            xt[:, sc * 512 : (sc + 1) * 512],
                            start=(e == 0),
                            stop=(e == 7),
                        )
            for fold in range(2):
                for sc in range(4):
                    nc.vector.tensor_copy(
                        kT[:, fold * S + sc * 512 : fold * S + (sc + 1) * 512],
                        psK[:, (fold * 4 + sc) * 512 : (fold * 4 + sc + 1) * 512],
                    )

        # ---- phase 2: v projection (natural layout [s, f]) ----
        # x already staged in xvAll; one accumulation group per PSUM bank
        with tc.tile_pool(name="psV", bufs=2, space="PSUM") as psvp:
            for sc in range(4):
                pvs = [
                    psvp.tile([128, 256], F32, name=f"pv{sub}", tag=f"psV{sub}")
                    for sub in range(4)
                ]
                for e in range(8):
                    for sub in range(4):
                        nc.tensor.matmul(
                            pvs[sub][:],
                            xvAll[:, e * S + sc * 512 + sub * 128 : e * S + sc * 512 + (sub + 1) * 128],
                            wv[:, e * F : (e + 1) * F],
                            start=(e == 0),
                            stop=(e == 7),
                        )
                for sub in range(4):
                    nc.vector.tensor_copy(
                        vv[:, sc * 1024 + sub * 256 : sc * 1024 + (sub + 1) * 256],
                        pvs[sub][:],
                    )

        # ---- phase 2.5: q projection for all four slabs ----
        with tc.tile_pool(name="psQ", bufs=2, space="PSUM") as psqp:
            for sc4 in range(4):
                pqs = [
                    psqp.tile([128, 512], F32, name=f"pq{fold}", tag=f"psQ{fold}")
                    for fold in range(2)
                ]
                for e in range(8):
                    for fold in range(2):
                        nc.tensor.matmul(
                            pqs[fold][:],
                            wq[:, e * F + fold * 128 : e * F + fold * 128 + 128],
                            xqAll[:, e * S + sc4 * 512 : e * S + (sc4 + 1) * 512],
                            start=(e == 0),
                            stop=(e == 7),
                        )
                for fold in range(2):
                    nc.scalar.copy(
                        qT[:, fold * S + sc4 * 512 : fold * S + (sc4 + 1) * 512],
                        pqs[fold][:],
                    )
        xbig.__exit__(None, None, None)

        # ---- phase 3: merged attention pipeline + output projection ----
        # PSUM budget (8 banks): psH [128,512]x5 = 5, po x1 = 1, accs x1 = 1,
        # acco x1 = 1.  All cross-engine consumers lag their producers by two
        # j-steps so the PE issue stream never carries an unsatisfied wait.
        # The reciprocal broadcast goes through DRAM + replicating DMAs on the
        # gpsimd SWDGE queue (FIFO-ordered), so the probability multiplies are
        # all-bf16 SBUF ops (DVE 2x mode) and the PE sheds the 224 broadcast
        # matmuls.
        psHp = ctx.enter_context(tc.tile_pool(name="psH", bufs=2, space="PSUM"))
        pop = ctx.enter_context(tc.tile_pool(name="poP", bufs=1, space="PSUM"))
        accp = ctx.enter_context(tc.tile_pool(name="accP", bufs=2, space="PSUM"))
        accop = ctx.enter_context(tc.tile_pool(name="accoP", bufs=1, space="PSUM"))
        expp = ctx.enter_context(tc.tile_pool(name="expS", bufs=4))
        btap = ctx.enter_context(tc.tile_pool(name="btA", bufs=2))
        ptp = ctx.enter_context(tc.tile_pool(name="pt", bufs=6))
        rcpp = ctx.enter_context(tc.tile_pool(name="rcp", bufs=2))
        attp = ctx.enter_context(tc.tile_pool(name="att", bufs=4))
        outp = ctx.enter_context(tc.tile_pool(name="outsb", bufs=2))

        def bcast_dma(scr, bta, h, g, j0, nj):
            """Replicate scr rows 32h+2j+g (j in [j0,j0+nj)) across dst
            partition group g of bta, one row per 512-wide j column block."""
            src = (
                scr.ap()[32 * h : 32 * h + 32, :]
                .rearrange("(j g) c -> g j c", g=2)[g][j0 : j0 + nj]
                .unsqueeze(0)
                .to_broadcast([64, nj, 512])
            )
            dst = bta[g * 64 : (g + 1) * 64, j0 * 512 : (j0 + nj) * 512].rearrange(
                "p (j c) -> p j c", c=512
            )
            nc.sync.dma_start(dst, src)

        def stream_A(t, f, ui):
            """Scores + exp + row sums for head pair (2f, 2f+1) of slab t,
            ending with the reciprocal + DRAM bounce + first broadcast DMAs.

            Returns (steps, shared) where shared collects the tiles stream_B
            needs.  Row sums lag scores by one j-step.
            """
            npt = NP_T[t]
            lo = LO[t]
            scr = scr_d[ui % 2]
            shared = {}

            def alloc(_k=0):
                shared["expS0"] = expp.tile([128, MAXP * QS], BF16, name="expS0", tag="expS")
                shared["expS1"] = expp.tile([128, MAXP * QS], BF16, name="expS1", tag="expS")

            def scores(k):
                # one macro = j pair (2k, 2k+1); one [128,1024] exp per head
                if k == 0:
                    alloc()
                for h, base in ((0, 0), (1, 64)):
                    ps = psHp.tile([128, 1024], F32, name="psH", tag="psH")
                    for jj in range(2):
                        c0 = lo + 2 * (2 * k + jj)
                        nc.tensor.matmul(
                            ps[:, jj * 512 : (jj + 1) * 512],
                            kT[base : base + 64, f * S + c0 * 64 : f * S + c0 * 64 + 128],
                            qT[base : base + 64, f * S + t * QS : f * S + (t + 1) * QS],
                            start=True,
                            stop=True,
                        )
                    nc.scalar.activation(
                        shared["expS0" if h == 0 else "expS1"][
                            :, 2 * k * QS : (2 * k + 2) * QS
                        ],
                        ps[:],
                        EXP,
                    )

            def rowsums(j):
                if j == 0:
                    shared["accs"] = accp.tile([128, 512], F32, name="accs", tag="accP")
                accs = shared["accs"]
                for h, expS in ((0, shared["expS0"]), (1, shared["expS1"])):
                    nc.tensor.matmul(
                        accs[32 * h : 32 * h + 32, :],
                        selb[:, j * 32 : (j + 1) * 32],
                        expS[:, j * QS : (j + 1) * QS],
                        start=(j == 0),
                        stop=(j == npt - 1),
                        skip_group_check=True,
                    )

            def recip_bounce():
                rowsums(npt - 2)
                rowsums(npt - 1)
                rc = rcpp.tile([64, 512], BF16, name="rc", tag="rcp")
                rs1 = rcpp.tile([64, 512], F32, name="rs1", tag="rcs1", bufs=1)
                rs2 = rcpp.tile([64, 512], F32, name="rs2", tag="rcs2", bufs=1)
                nc.vector.reciprocal_approx_accurate(rs2[:], shared["accs"][0:64, :], rs1[:])
                nc.vector.tensor_mul(rc[:], rs2[:], vm[:, t * QS : (t + 1) * QS])
                nc.sync.dma_start(scr.ap(), rc[:])
                shared["bt"] = [
                    btap.tile([128, MAXP * QS], BF16, name=f"bta{h}", tag=f"btA{h}")
                    for h in range(2)
                ]
                # issue the full broadcast now; the gpsimd queue drains it
                # while the next unit's scores run
                for h in range(2):
                    for g in range(2):
                        bcast_dma(scr, shared["bt"][h], h, g, 0, min(4, npt))
                for h in range(2):
                    for g in range(2):
                        bcast_dma(scr, shared["bt"][h], h, g, 4, npt - 4)

            mA = npt // 2
            steps = []
            for k in range(mA):
                steps.append(
                    lambda k=k: (
                        scores(k),
                        k > 0 and (rowsums(2 * k - 2), rowsums(2 * k - 1)),
                    )
                )
            steps.append(recip_bounce)
            return steps, shared

        def stream_B(t, f, ui, shared, attn_t):
            """Probability multiplies + attn@V for the unit A just finished.

            The reciprocal broadcast tensors arrive by DMA (issued in A's tail
            and in early B steps); pt-muls are all-bf16 SBUF DVE ops.  V
            matmuls lag the pt-mul by two j-steps.
            """
            npt = NP_T[t]
            lo = LO[t]
            scr = scr_d[ui % 2]
            st = {}

            def ptmul(k):
                pts = []
                for h in range(2):
                    expS = shared["expS0" if h == 0 else "expS1"]
                    ptt = ptp.tile([128, 1024], BF16, name="ptt", tag="pt")
                    nc.vector.tensor_mul(
                        ptt[:],
                        expS[:, 2 * k * QS : (2 * k + 2) * QS],
                        shared["bt"][h][:, 2 * k * QS : (2 * k + 2) * QS],
                    )
                    pts.append(ptt)
                st[("pt", k)] = pts

            def vmm(k):
                if k == 0:
                    st["acco"] = accop.tile([128, 512], F32, name="acco", tag="accoP")
                acco = st["acco"]
                pts = st.pop(("pt", k))
                for jj in range(2):
                    j = 2 * k + jj
                    cp = lo // 2 + j
                    for h in range(2):
                        nc.tensor.matmul(
                            acco[64 * h : 64 * h + 64, :],
                            vv[:, cp * F + (2 * f + h) * 64 : cp * F + (2 * f + h) * 64 + 64],
                            pts[h][:, jj * 512 : (jj + 1) * 512],
                            start=(j == 0),
                            stop=(j == npt - 1),
                            skip_group_check=True,
                        )

            mB = npt // 2
            steps = []
            for k in range(mB):
                steps.append(lambda k=k: (ptmul(k), k > 0 and vmm(k - 1)))
            steps.append(lambda: (vmm(mB - 1), nc.vector.tensor_copy(attn_t[:], st["acco"][:])))
            return steps

        def stream_C(t, atts, pad=True):
            """Output projection of slab t, dripped 2 matmuls per macro step."""
            st = {}

            def piece(sc2, eh):
                if eh == 0:
                    st["ob"] = outp.tile([128, 1024], F32, name="ob", tag="outsb")
                if (2 * sc2 + eh) % 2 == 0:
                    po = pop.tile([128, 512], F32, name="po", tag="poP")
                else:
                    po = accp.tile([128, 512], F32, name="po2", tag="accP")
                for f in range(2):
                    nc.tensor.matmul(
                        po[:],
                        atts[f][:, sc2 * 128 : sc2 * 128 + 128],
                        wo[:, f * E + eh * 512 : f * E + eh * 512 + 512],
                        start=(f == 0),
                        stop=(f == 1),
                    )
                nc.vector.tensor_copy(st["ob"][:, eh * 512 : (eh + 1) * 512], po[:])
                if eh == 1:
                    row = (4 * t + sc2) * 128
                    nc.sync.dma_start(out_d.ap()[row : row + 128, :], st["ob"][:])

            steps = []
            for sc2 in range(4):
                for eh in range(2):
                    steps.append(lambda sc2=sc2, eh=eh: piece(sc2, eh))
                    if pad:
                        steps.append(lambda: None)  # half rate: po shares btP slots
            return steps

        def merge(streams):
            for k in range(max(len(s) for s in streams)):
                for s in streams:
                    if k < len(s):
                        s[k]()

        units = [(t, f) for t in range(T_SLABS) for f in range(2)]
        atts_by_t = {t: [] for t in range(T_SLABS)}
        pending_B = None
        for i, (t, f) in enumerate(units):
            sA, shared = stream_A(t, f, i)
            streams = [sA]
            if pending_B is not None:
                streams.append(pending_B)
            if f == 1 and t >= 1:
                streams.append(stream_C(t - 1, atts_by_t[t - 1], pad=False))
            merge(streams)
            attn_t = attp.tile([128, 512], BF16, name="attn_t", tag="att")
            atts_by_t[t].append(attn_t)
            pending_B = stream_B(t, f, i, shared, attn_t)
        merge([pending_B])
        merge([stream_C(T_SLABS - 1, atts_by_t[T_SLABS - 1], pad=False)])

    nc.compile()
    return nc


_NC_CACHE = []


def _get_nc():
    if not _NC_CACHE:
        _NC_CACHE.append(build_nc())
    return _NC_CACHE[0]


def _host_consts():
    selc = np.zeros((128, MAXP * 32), np.float32)
    for k in range(128):
        for j in range(MAXP):
            selc[k, j * 32 + 2 * j + k // 64] = 1.0
    vmask = np.zeros((64, T_SLABS * QS), np.float32)
    for t in range(T_SLABS):
        for m in range(2 * NP_T[t]):
            c = LO[t] + m
            for qb in range(QS // BLK):
                r = 8 * t + qb
                if abs(r - c) <= BAND:
                    vmask[m, t * QS + qb * 64 : t * QS + (qb + 1) * 64] = 1.0
                    vmask[32 + m, t * QS + qb * 64 : t * QS + (qb + 1) * 64] = 1.0
    return selc.astype(BFD), vmask.astype(BFD)


def build_in_maps(query, key, value, Wq, Wk, Wv, Wo):
    query = np.asarray(query, np.float32)
    key = np.asarray(key, np.float32)
    value = np.asarray(value, np.float32)
    Wq = np.asarray(Wq, np.float32)
    Wk = np.asarray(Wk, np.float32)
    Wv = np.asarray(Wv, np.float32)
    Wo = np.asarray(Wo, np.float32)

    selc, vmask = _host_consts()
    xs = [np.ascontiguousarray(a[b].T).astype(BFD) for a in (query, key, value) for b in range(B)]
    in_maps = []
    for c in range(NCORES):
        b, g = divmod(c, HPC)
        fs = slice(F * g, F * (g + 1))
        in_maps.append(
            {
                "xqT": xs[0 + b],
                "xkT": xs[2 + b],
                "xvT": xs[4 + b],
                "wqT": np.ascontiguousarray((Wq[fs, :] * SCALE).T).astype(BFD),
                "wkT": np.ascontiguousarray(Wk[fs, :].T).astype(BFD),
                "wvT": np.ascontiguousarray(Wv[fs, :].T).astype(BFD),
                "woT": np.ascontiguousarray(Wo[:, fs].T).astype(BFD),
                "selc": selc,
                "vmask": vmask,
            }
        )
    return in_maps


def kernel(query, key, value, Wq, Wk, Wv, Wo):
    nc = _get_nc()
    in_maps = build_in_maps(query, key, value, Wq, Wk, Wv, Wo)
    res = bass_utils.run_bass_kernel_spmd(nc, in_maps, core_ids=list(range(NCORES)))
    out = np.zeros((B, S, E), np.float32)
    for c in range(NCORES):
        b = c // HPC
        out[b] += res.results[c]["out"]
    return out


# revision 46
# speedup vs baseline: 1.1578x; 1.1484x over previous
"""Block-sparse (banded) attention kernel for Trainium2, 8 NeuronCores.

Sharding: data-parallel over batch (2) x tensor-parallel over heads
(16 heads -> 4 per core).  Each core computes its 4 heads' Q/K/V
projections, banded block attention (|r-c| <= 15 blocks, per-block
softmax), and a partial output projection; the host sums the 4 partial
outputs per batch element.

All matmul operands are bf16 (PSUM accumulation stays fp32).  Heads are
processed in pairs per fold; scores / row-sum / broadcast / attn@V
matmuls run as concurrent tile_position'd pairs.  Phase 3 merges the
scores pipeline of unit u with the value pipeline of unit u-1 and the
output projection of the previous slab at macro-step granularity so the
in-order PE queue never chains behind ACT/DVE latency.

Self-contained: hardcodes all shapes; only needs the concourse tree that
the environment already puts on sys.path.
"""

import sys

for _p in ("/opt/trn_rl_repo",):
    if _p not in sys.path:
        sys.path.insert(0, _p)

from contextlib import ExitStack

import numpy as np
import ml_dtypes

import concourse.bacc as bacc
import concourse.tile as tile
from concourse import bass_utils, mybir

F32 = mybir.dt.float32
BF16 = mybir.dt.bfloat16
EXP = mybir.ActivationFunctionType.Exp

B, S, E = 2, 2048, 1024
H, HD, BLK = 16, 64, 64
NB = S // BLK  # 32 blocks
NCORES = 8
HPC = 4  # heads per core
F = HPC * HD  # 256 local features
BAND = 15
SCALE = HD ** -0.5
BFD = ml_dtypes.bfloat16

# per r8-slab (8 query blocks, q=512) column-block ranges, even-extended
T_SLABS = 4
QS = 512  # q extent per slab
LO = []
NP_T = []
for _t in range(T_SLABS):
    lo = max(0, 8 * _t - BAND)
    hi = min(NB - 1, 8 * _t + 7 + BAND)
    if (hi - lo + 1) % 2 == 1:
        if lo > 0:
            lo -= 1
        else:
            hi += 1
    LO.append(lo)
    NP_T.append((hi - lo + 1) // 2)
MAXP = max(NP_T)  # 16 pairs


def build_nc():
    nc = bacc.Bacc("TRN2", target_bir_lowering=False, debug=False)

    xq_d = nc.dram_tensor("xqT", [E, S], BF16, kind="ExternalInput")
    xk_d = nc.dram_tensor("xkT", [E, S], BF16, kind="ExternalInput")
    xv_d = nc.dram_tensor("xvT", [E, S], BF16, kind="ExternalInput")
    wq_d = nc.dram_tensor("wqT", [E, F], BF16, kind="ExternalInput")
    wk_d = nc.dram_tensor("wkT", [E, F], BF16, kind="ExternalInput")
    wv_d = nc.dram_tensor("wvT", [E, F], BF16, kind="ExternalInput")
    wo_d = nc.dram_tensor("woT", [F, E], BF16, kind="ExternalInput")
    sel_d = nc.dram_tensor("selc", [128, MAXP * 32], BF16, kind="ExternalInput")
    vm_d = nc.dram_tensor("vmask", [64, T_SLABS * QS], BF16, kind="ExternalInput")
    out_d = nc.dram_tensor("out", [S, E], F32, kind="ExternalOutput")
    # ping-pong DRAM staging for the reciprocal broadcast (DMA partition
    # replication needs a DRAM source: SBUF APs can't have zero-step
    # partition dims)
    scr_d = [
        nc.dram_tensor(f"rcscr{i}", [64, 512], BF16, kind="Internal") for i in range(2)
    ]

    with tile.TileContext(nc) as tc, ExitStack() as ctx, nc.allow_low_precision(
        reason="bf16 pipeline; fp32 PSUM accumulation throughout"
    ):
        pers = ctx.enter_context(tc.tile_pool(name="pers", bufs=1))
        qT = pers.tile([128, 2 * S], BF16, tag="qT")
        kT = pers.tile([128, 2 * S], BF16, tag="kT")
        vv = pers.tile([128, 16 * F], BF16, tag="vv")
        wq = pers.tile([128, 8 * F], BF16, tag="wq")
        wk = pers.tile([128, 8 * F], BF16, tag="wk")
        wv = pers.tile([128, 8 * F], BF16, tag="wv")
        wo = pers.tile([128, 2 * E], BF16, tag="wo")
        selb = pers.tile([128, MAXP * 32], BF16, tag="selb")
        vm = pers.tile([64, T_SLABS * QS], BF16, tag="vm")

        # k-projection weights first: phase 1 is on the critical path
        nc.sync.dma_start(
            wk[:].rearrange("p (c f) -> p c f", c=8),
            wk_d.ap().rearrange("(c p) f -> p c f", p=128),
        )
        # remaining weights/constants arrive via gpsimd (SWDGE) so they don't
        # queue ahead of the phase-1/2 x-tile loads on the sync ring
        nc.gpsimd.dma_start(
            wv[:].rearrange("p (c f) -> p c f", c=8),
            wv_d.ap().rearrange("(c p) f -> p c f", p=128),
        )
        nc.gpsimd.dma_start(
            wq[:].rearrange("p (c f) -> p c f", c=8),
            wq_d.ap().rearrange("(c p) f -> p c f", p=128),
        )
        nc.gpsimd.dma_start(
            wo[:].rearrange("p (c e) -> p c e", c=2),
            wo_d.ap().rearrange("(c p) e -> p c e", p=128),
        )
        nc.gpsimd.dma_start(selb[:], sel_d.ap())
        nc.gpsimd.dma_start(vm[:], vm_d.ap())

        # big staging tiles for xv/xq: loaded as full [128, 2048] rows (4KB
        # per-partition DMA lines) while phase 1 computes; freed before the
        # attention pools open
        xbig = tc.tile_pool(name="xbig", bufs=1)
        xbp = xbig.__enter__()
        xvAll = xbp.tile([128, 8 * S], BF16, tag="xvAll")
        xqAll = xbp.tile([128, 8 * S], BF16, tag="xqAll")

        # ---- phase 1: k projection (kT layout [f, s]) ----
        with tc.tile_pool(name="xk", bufs=2) as xkp, tc.tile_pool(
            name="psK", bufs=1, space="PSUM"
        ) as pskp:
            psK = pskp.tile([128, 4096], F32)
            for e in range(8):
                xt = xkp.tile([128, S], BF16, tag="xk")
                nc.sync.dma_start(xt[:], xk_d.ap()[e * 128 : (e + 1) * 128, :])
                # xv/xq stream on the scalar/vector queues so the sync ring
                # feeds phase-1 xk tiles back-to-back
                nc.scalar.dma_start(
                    xvAll[:, e * S : (e + 1) * S], xv_d.ap()[e * 128 : (e + 1) * 128, :]
                )
                nc.scalar.dma_start(
                    xqAll[:, e * S : (e + 1) * S], xq_d.ap()[e * 128 : (e + 1) * 128, :]
                )
                for fold in range(2):
                    for sc in range(4):
                        nc.tensor.matmul(
                            psK[:, (fold * 4 + sc) * 512 : (fold * 4 + sc + 1) * 512],
                            wk[:, e * F + fold * 128 : e * F + fold * 128 + 128],
                            xt[:, sc * 512 : (sc + 1) * 512],
                            start=(e == 0),
                            stop=(e == 7),
                        )
            for fold in range(2):
                for sc in range(4):
                    nc.vector.tensor_copy(
                        kT[:, fold * S + sc * 512 : fold * S + (sc + 1) * 512],
                        psK[:, (fold * 4 + sc) * 512 : (fold * 4 + sc + 1) * 512],
                    )

        # ---- phase 2: v projection (natural layout [s, f]) ----
        # x already staged in xvAll; one accumulation group per PSUM bank
        with tc.tile_pool(name="psV", bufs=2, space="PSUM") as psvp:
            for sc in range(4):
                pvs = [
                    psvp.tile([128, 256], F32, name=f"pv{sub}", tag=f"psV{sub}")
                    for sub in range(4)
                ]
                for e in range(8):
                    for sub in range(4):
                        nc.tensor.matmul(
                            pvs[sub][:],
                            xvAll[:, e * S + sc * 512 + sub * 128 : e * S + sc * 512 + (sub + 1) * 128],
                            wv[:, e * F : (e + 1) * F],
                            start=(e == 0),
                            stop=(e == 7),
                        )
                for sub in range(4):
                    nc.vector.tensor_copy(
                        vv[:, sc * 1024 + sub * 256 : sc * 1024 + (sub + 1) * 256],
                        pvs[sub][:],
                    )

        # ---- phase 2.5: q projection for all four slabs ----
        with tc.tile_pool(name="psQ", bufs=2, space="PSUM") as psqp:
            for sc4 in range(4):
                pqs = [
                    psqp.tile([128, 512], F32, name=f"pq{fold}", tag=f"psQ{fold}")
                    for fold in range(2)
                ]
                for e in range(8):
                    for fold in range(2):
                        nc.tensor.matmul(
                            pqs[fold][:],
                            wq[:, e * F + fold * 128 : e * F + fold * 128 + 128],
                            xqAll[:, e * S + sc4 * 512 : e * S + (sc4 + 1) * 512],
                            start=(e == 0),
                            stop=(e == 7),
                        )
                for fold in range(2):
                    nc.scalar.copy(
                        qT[:, fold * S + sc4 * 512 : fold * S + (sc4 + 1) * 512],
                        pqs[fold][:],
                    )
        xbig.__exit__(None, None, None)

        # ---- phase 3: merged attention pipeline + output projection ----
        # PSUM budget (8 banks): psH [128,512]x5 = 5, po x1 = 1, accs x1 = 1,
        # acco x1 = 1.  All cross-engine consumers lag their producers by two
        # j-steps so the PE issue stream never carries an unsatisfied wait.
        # The reciprocal broadcast goes through DRAM + replicating DMAs on the
        # gpsimd SWDGE queue (FIFO-ordered), so the probability multiplies are
        # all-bf16 SBUF ops (DVE 2x mode) and the PE sheds the 224 broadcast
        # matmuls.
        psHp = ctx.enter_context(tc.tile_pool(name="psH", bufs=2, space="PSUM"))
        pop = ctx.enter_context(tc.tile_pool(name="poP", bufs=1, space="PSUM"))
        accp = ctx.enter_context(tc.tile_pool(name="accP", bufs=2, space="PSUM"))
        accop = ctx.enter_context(tc.tile_pool(name="accoP", bufs=1, space="PSUM"))
        expp = ctx.enter_context(tc.tile_pool(name="expS", bufs=4))
        btap = ctx.enter_context(tc.tile_pool(name="btA", bufs=2))
        ptp = ctx.enter_context(tc.tile_pool(name="pt", bufs=6))
        rcpp = ctx.enter_context(tc.tile_pool(name="rcp", bufs=2))
        attp = ctx.enter_context(tc.tile_pool(name="att", bufs=4))
        outp = ctx.enter_context(tc.tile_pool(name="outsb", bufs=2))

        def bcast_dma(scr, bta, h, g, j0, nj):
            """Replicate scr rows 32h+2j+g (j in [j0,j0+nj)) across dst
            partition group g of bta, one row per 512-wide j column block."""
            src = (
                scr.ap()[32 * h : 32 * h + 32, :]
                .rearrange("(j g) c -> g j c", g=2)[g][j0 : j0 + nj]
                .unsqueeze(0)
                .to_broadcast([64, nj, 512])
            )
            dst = bta[g * 64 : (g + 1) * 64, j0 * 512 : (j0 + nj) * 512].rearrange(
                "p (j c) -> p j c", c=512
            )
            nc.sync.dma_start(dst, src)

        def stream_A(t, f, ui):
            """Scores + exp + row sums for head pair (2f, 2f+1) of slab t,
            ending with the reciprocal + DRAM bounce + first broadcast DMAs.

            Returns (steps, shared) where shared collects the tiles stream_B
            needs.  Row sums lag scores by one j-step.
            """
            npt = NP_T[t]
            lo = LO[t]
            scr = scr_d[ui % 2]
            shared = {}

            def alloc(_k=0):
                shared["expS0"] = expp.tile([128, MAXP * QS], BF16, name="expS0", tag="expS")
                shared["expS1"] = expp.tile([128, MAXP * QS], BF16, name="expS1", tag="expS")

            def scores(k):
                # one macro = j pair (2k, 2k+1); one [128,1024] exp per head
                if k == 0:
                    alloc()
                for h, base in ((0, 0), (1, 64)):
                    ps = psHp.tile([128, 1024], F32, name="psH", tag="psH")
                    for jj in range(2):
                        c0 = lo + 2 * (2 * k + jj)
                        nc.tensor.matmul(
                            ps[:, jj * 512 : (jj + 1) * 512],
                            kT[base : base + 64, f * S + c0 * 64 : f * S + c0 * 64 + 128],
                            qT[base : base + 64, f * S + t * QS : f * S + (t + 1) * QS],
                            start=True,
                            stop=True,
                        )
                    nc.scalar.activation(
                        shared["expS0" if h == 0 else "expS1"][
                            :, 2 * k * QS : (2 * k + 2) * QS
                        ],
                        ps[:],
                        EXP,
                    )

            def rowsums(j):
                if j == 0:
                    shared["accs"] = accp.tile([128, 512], F32, name="accs", tag="accP")
                accs = shared["accs"]
                for h, expS in ((0, shared["expS0"]), (1, shared["expS1"])):
                    nc.tensor.matmul(
                        accs[32 * h : 32 * h + 32, :],
                        selb[:, j * 32 : (j + 1) * 32],
                        expS[:, j * QS : (j + 1) * QS],
                        start=(j == 0),
                        stop=(j == npt - 1),
                        skip_group_check=True,
                    )

            def recip_bounce():
                rowsums(npt - 2)
                rowsums(npt - 1)
                rc = rcpp.tile([64, 512], BF16, name="rc", tag="rcp")
                rs1 = rcpp.tile([64, 512], F32, name="rs1", tag="rcs1", bufs=1)
                rs2 = rcpp.tile([64, 512], F32, name="rs2", tag="rcs2", bufs=1)
                nc.vector.reciprocal_approx_accurate(rs2[:], shared["accs"][0:64, :], rs1[:])
                nc.vector.tensor_mul(rc[:], rs2[:], vm[:, t * QS : (t + 1) * QS])
                nc.sync.dma_start(scr.ap(), rc[:])
                shared["bt"] = [
                    btap.tile([128, MAXP * QS], BF16, name=f"bta{h}", tag=f"btA{h}")
                    for h in range(2)
                ]
                # issue the full broadcast now; the gpsimd queue drains it
                # while the next unit's scores run
                for h in range(2):
                    for g in range(2):
                        bcast_dma(scr, shared["bt"][h], h, g, 0, min(4, npt))
                for h in range(2):
                    for g in range(2):
                        bcast_dma(scr, shared["bt"][h], h, g, 4, npt - 4)

            mA = npt // 2
            steps = []
            for k in range(mA):
                steps.append(
                    lambda k=k: (
                        scores(k),
                        k > 0 and (rowsums(2 * k - 2), rowsums(2 * k - 1)),
                    )
                )
            steps.append(recip_bounce)
            return steps, shared

        def stream_B(t, f, ui, shared, attn_t):
            """Probability multiplies + attn@V for the unit A just finished.

            The reciprocal broadcast tensors arrive by DMA (issued in A's tail
            and in early B steps); pt-muls are all-bf16 SBUF DVE ops.  V
            matmuls lag the pt-mul by two j-steps.
            """
            npt = NP_T[t]
            lo = LO[t]
            scr = scr_d[ui % 2]
            st = {}

            def ptmul(k):
                pts = []
                for h in range(2):
                    expS = shared["expS0" if h == 0 else "expS1"]
                    ptt = ptp.tile([128, 1024], BF16, name="ptt", tag="pt")
                    nc.vector.tensor_mul(
                        ptt[:],
                        expS[:, 2 * k * QS : (2 * k + 2) * QS],
                        shared["bt"][h][:, 2 * k * QS : (2 * k + 2) * QS],
                    )
                    pts.append(ptt)
                st[("pt", k)] = pts

            def vmm(k):
                if k == 0:
                    st["acco"] = accop.tile([128, 512], F32, name="acco", tag="accoP")
                acco = st["acco"]
                pts = st.pop(("pt", k))
                for jj in range(2):
                    j = 2 * k + jj
                    cp = lo // 2 + j
                    for h in range(2):
                        nc.tensor.matmul(
                            acco[64 * h : 64 * h + 64, :],
                            vv[:, cp * F + (2 * f + h) * 64 : cp * F + (2 * f + h) * 64 + 64],
                            pts[h][:, jj * 512 : (jj + 1) * 512],
                            start=(j == 0),
                            stop=(j == npt - 1),
                            skip_group_check=True,
                        )

            mB = npt // 2
            steps = []
            for k in range(mB):
                steps.append(lambda k=k: (ptmul(k), k > 0 and vmm(k - 1)))
            steps.append(lambda: (vmm(mB - 1), nc.vector.tensor_copy(attn_t[:], st["acco"][:])))
            return steps

        def stream_C(t, atts, pad=True):
            """Output projection of slab t, dripped 2 matmuls per macro step."""
            st = {}

            def piece(sc2, eh):
                if eh == 0:
                    st["ob"] = outp.tile([128, 1024], F32, name="ob", tag="outsb")
                po = pop.tile([128, 512], F32, name="po", tag="poP")
                for f in range(2):
                    nc.tensor.matmul(
                        po[:],
                        atts[f][:, sc2 * 128 : sc2 * 128 + 128],
                        wo[:, f * E + eh * 512 : f * E + eh * 512 + 512],
                        start=(f == 0),
                        stop=(f == 1),
                    )
                nc.vector.tensor_copy(st["ob"][:, eh * 512 : (eh + 1) * 512], po[:])
                if eh == 1:
                    row = (4 * t + sc2) * 128
                    nc.sync.dma_start(out_d.ap()[row : row + 128, :], st["ob"][:])

            steps = []
            for sc2 in range(4):
                for eh in range(2):
                    steps.append(lambda sc2=sc2, eh=eh: piece(sc2, eh))
                    if pad:
                        steps.append(lambda: None)  # half rate: po shares btP slots
            return steps

        def merge(streams):
            for k in range(max(len(s) for s in streams)):
                for s in streams:
                    if k < len(s):
                        s[k]()

        units = [(t, f) for t in range(T_SLABS) for f in range(2)]
        atts_by_t = {t: [] for t in range(T_SLABS)}
        pending_B = None
        for i, (t, f) in enumerate(units):
            sA, shared = stream_A(t, f, i)
            streams = [sA]
            if pending_B is not None:
                streams.append(pending_B)
            if f == 1 and t >= 1:
                streams.append(stream_C(t - 1, atts_by_t[t - 1], pad=False))
            merge(streams)
            attn_t = attp.tile([128, 512], BF16, name="attn_t", tag="att")
            atts_by_t[t].append(attn_t)
            pending_B = stream_B(t, f, i, shared, attn_t)
        merge([pending_B])
        merge([stream_C(T_SLABS - 1, atts_by_t[T_SLABS - 1], pad=False)])

    nc.compile()
    return nc


_NC_CACHE = []


def _get_nc():
    if not _NC_CACHE:
        _NC_CACHE.append(build_nc())
    return _NC_CACHE[0]


def _host_consts():
    selc = np.zeros((128, MAXP * 32), np.float32)
    for k in range(128):
        for j in range(MAXP):
            selc[k, j * 32 + 2 * j + k // 64] = 1.0
    vmask = np.zeros((64, T_SLABS * QS), np.float32)
    for t in range(T_SLABS):
        for m in range(2 * NP_T[t]):
            c = LO[t] + m
            for qb in range(QS // BLK):
                r = 8 * t + qb
                if abs(r - c) <= BAND:
                    vmask[m, t * QS + qb * 64 : t * QS + (qb + 1) * 64] = 1.0
                    vmask[32 + m, t * QS + qb * 64 : t * QS + (qb + 1) * 64] = 1.0
    return selc.astype(BFD), vmask.astype(BFD)


def build_in_maps(query, key, value, Wq, Wk, Wv, Wo):
    query = np.asarray(query, np.float32)
    key = np.asarray(key, np.float32)
    value = np.asarray(value, np.float32)
    Wq = np.asarray(Wq, np.float32)
    Wk = np.asarray(Wk, np.float32)
    Wv = np.asarray(Wv, np.float32)
    Wo = np.asarray(Wo, np.float32)

    selc, vmask = _host_consts()
    xs = [np.ascontiguousarray(a[b].T).astype(BFD) for a in (query, key, value) for b in range(B)]
    in_maps = []
    for c in range(NCORES):
        b, g = divmod(c, HPC)
        fs = slice(F * g, F * (g + 1))
        in_maps.append(
            {
                "xqT": xs[0 + b],
                "xkT": xs[2 + b],
                "xvT": xs[4 + b],
                "wqT": np.ascontiguousarray((Wq[fs, :] * SCALE).T).astype(BFD),
                "wkT": np.ascontiguousarray(Wk[fs, :].T).astype(BFD),
                "wvT": np.ascontiguousarray(Wv[fs, :].T).astype(BFD),
                "woT": np.ascontiguousarray(Wo[:, fs].T).astype(BFD),
                "selc": selc,
                "vmask": vmask,
            }
        )
    return in_maps


def kernel(query, key, value, Wq, Wk, Wv, Wo):
    nc = _get_nc()
    in_maps = build_in_maps(query, key, value, Wq, Wk, Wv, Wo)
    res = bass_utils.run_bass_kernel_spmd(nc, in_maps, core_ids=list(range(NCORES)))
    out = np.zeros((B, S, E), np.float32)
    for c in range(NCORES):
        b = c // HPC
        out[b] += res.results[c]["out"]
    return out
